# revision 37
# baseline (speedup 1.0000x reference)
# Paged sparse attention (GQA, block-masked new tokens) on 8 TRN2 NeuronCores.
#
# Sharding: tensor-parallel over the 8 KV heads (one KV head + its 4 Q heads
# per core). Every core sees all 8 sequences, so the compiled schedule
# (derived from page_tables/context_lens, identical across cores) is SPMD.
#
# Orientation: scores are computed TRANSPOSED (S^T[t, sg] per 128-row
# t-block, K^T-stationary, Q^T-moving), so the exp (ACT) writes P^T directly
# in the layout the PV matmul consumes — no probability transposes anywhere.
# Masking folds into the exp's per-partition bias (host-precomputed -1e30
# rows for the partial page / 32-alignment gap / tail pad). The softmax
# denominator comes from an extra matmul with an all-ones stationary matrix,
# which leaves the per-sg denominator replicated across all 128 PSUM
# partitions — the normalization is then a fused (OUT^T * 1/denom -> bf16)
# DVE pass, transposed back to [sg, d] by the host during the final gather.
#
# The host pre-assembles, per sequence, contiguous zero-padded K^T / V / Q^T
# buffers (page-table gather, 32-alignment gap, new tokens, tail pad all
# resolved in numpy), so every device load is one large contiguous DMA —
# no on-device DMA transposes, no gather runs, no memsets.
#
# The whole kernel is emitted as ONE flat software pipeline over t-blocks
# across all sequences: scores(i) issue ahead of PV(i-1), so at sequence
# boundaries the next sequence's first exp is never stuck behind the
# previous sequence's tail matmuls. outt/dent PSUM banks swap roles between
# consecutive sequences so the first PV of a sequence only waits on the
# previous sequence's (cheap) denominator-reciprocal read, not its full
# normalization.
#
# The block-causal mask for new tokens reduces (with sg = s*4+g ordering) to
# a suffix of valid sg columns per t-block (plus a small intra-block
# staircase zeroed on the bf16 P^T), so invalid regions are simply never
# computed. The softmax denominator streams P^T through a ones-matmul; runs
# of up to 16 qmin-equal t-blocks are pre-summed on DVE (incremental
# left-leaning bf16 accumulator, <=2 adds per pipeline step) so the
# ones-matmul touches each column once per run. Sequences are processed
# 2nd-largest first and LARGEST last: the final sequence's long context
# phase absorbs every other sequence's deferred denominator/normalize/store
# work, and its last group is a single t-block, keeping the endgame chain
# (exp -> matmul -> reciprocal -> mul -> store) short.

import sys

sys.path.insert(0, "/opt/trn_rl_repo")

import ml_dtypes
import numpy as np

B = 8
S = 256
NUM_HEADS = 32
NUM_KV_HEADS = 8
G = NUM_HEADS // NUM_KV_HEADS  # 4
HD = 128
PAGE = 16
BLOCK = 32
MAX_PAGES = 128
C = MAX_PAGES * PAGE  # 2048
SCALE = 0.08838834764831845
SG = S * G  # 1024 q rows per (seq, kv head)
TMAX = C + S + 32  # worst-case padded length
NTBMAX = (TMAX + 127) // 128
NQT = SG // 128  # 8 q-tiles per seq

NEG = -1e30


def _schedule(context_lens: np.ndarray):
    """Per-seq schedule baked into the compiled kernel (same on all cores)."""
    desc = sorted(
        range(B),
        key=lambda b: -(((int(context_lens[b]) + 15) // 16 * 16 + 31) // 32 * 32),
    )
    # 2nd-largest first (covers load latency), then descending, with the
    # LARGEST last: its long context phase absorbs every other sequence's
    # deferred denominator/close work, so almost nothing piles up after
    # the final exp
    order = desc[1:] + desc[:1]
    seqs = []
    for b in range(B):
        ctx = int(context_lens[b])
        npg = (ctx + PAGE - 1) // PAGE
        ctxp = npg * PAGE
        ctxp32 = ((ctxp + 31) // 32) * 32  # 32-align the new-token region
        ttot = ctxp32 + S
        ntb = (ttot + 127) // 128
        tq = [ctxp32 + BLOCK * (i + 1) for i in range(NQT)]
        # first valid q-tile per t-block (valid sg columns = suffix)
        qmin = [next(i for i in range(NQT) if tq[i] > tb * 128) for tb in range(ntb)]

        def fully_valid(tb):
            # every t-row in the block is a real, unmasked token
            if (tb + 1) * 128 > ttot:
                return False
            return not (ctx < (tb + 1) * 128 and tb * 128 < ctxp32)

        # denominator groups: runs of qmin-equal t-blocks pre-summed on DVE.
        # Last seq: single final group = short endgame chain.
        maxrun = 16
        dgroups = []
        tb = 0
        while tb < ntb:
            n = 1
            while n < maxrun and tb + n < ntb and qmin[tb + n] == qmin[tb]:
                n += 1
            if b == order[-1] and tb + n == ntb and n > 1:
                n -= 1  # force the final group to be a single
            dgroups.append(tuple(range(tb, tb + n)))
            tb += n
        seqs.append(
            dict(
                ctx=ctx,
                ctxp=ctxp,
                ctxp32=ctxp32,
                ttot=ttot,
                ntb=ntb,
                tq=tq,
                qmin=qmin,
                dgroups=dgroups,
                valid=[fully_valid(tb) for tb in range(ntb)],
            )
        )
    return seqs, order


def _masks(seqs):
    """Host-precomputed per-partition exp bias: [128, B, NTBMAX] fp32.
    mask[p, b, tb] is added (post-scale) to scores of t-row tb*128+p:
    0 for valid rows, -1e30 for masked rows (partial page, 32-align gap,
    padded tail)."""
    m = np.zeros((B, 128, NTBMAX), np.float32)
    for b, sq in enumerate(seqs):
        valid = np.zeros((NTBMAX * 128,), bool)
        valid[: sq["ttot"]] = True
        valid[sq["ctx"] : sq["ctxp32"]] = False  # partial page + gap
        m[b][~valid.reshape(NTBMAX, 128).T] = NEG
    return np.ascontiguousarray(m.transpose(1, 0, 2))


def _build(nc, seqs, order):
    import concourse.mybir as mybir
    import concourse.tile as tile

    bf16 = mybir.dt.bfloat16
    f32 = mybir.dt.float32

    # host-assembled, per-seq contiguous + padded, transposed layouts
    kth = nc.dram_tensor("kth", [B, HD, NTBMAX * 128], bf16, kind="ExternalInput").ap()
    vth = nc.dram_tensor("vth", [B, 128, NTBMAX, HD], bf16, kind="ExternalInput").ap()
    qth = nc.dram_tensor("qth", [B, HD, SG], bf16, kind="ExternalInput").ap()
    mh = nc.dram_tensor("mh", [128, B, NTBMAX], f32, kind="ExternalInput").ap()
    # transposed output [b, d, sg] in bf16; the host normz-reindexes
    outh = nc.dram_tensor("outh", [B, HD, SG], bf16, kind="ExternalOutput").ap()

    KSPLIT = 128  # first seq: tiny first K^T tile so matmuls start early
    chunks = ((0, 4), (4, 8))

    with tile.TileContext(nc) as tc:
        with (
            tc.tile_pool(name="cst", bufs=1) as const_pool,
            tc.tile_pool(name="kt", bufs=3) as kt_pool,
            tc.tile_pool(name="vt", bufs=3) as v_pool,
            tc.tile_pool(name="qt", bufs=3) as qt_pool,
            tc.tile_pool(name="pt", bufs=3) as pt_pool,
            tc.tile_pool(name="ds", bufs=6) as dsum_pool,
            tc.tile_pool(name="ot", bufs=2) as out_pool,
            tc.tile_pool(name="ps_s", bufs=2, space="PSUM") as psum_s,
            tc.tile_pool(name="ps_a", bufs=1, space="PSUM") as psum_a,
            tc.tile_pool(name="ps_b", bufs=1, space="PSUM") as psum_b,
        ):
            ones_t = const_pool.tile([128, 128], bf16)
            nc.vector.memset(ones_t, 1.0)

            mask_all = const_pool.tile([128, B, NTBMAX], f32)

            load_tiles = {}

            def emit_loads(b, first=False):
                sq = seqs[b]
                ntb = sq["ntb"]

                # Q^T first on the gpsimd queue: the first score matmul
                # needs only qt + the first K^T tile
                qt = qt_pool.tile([128, SG], bf16, tag="qt")
                if first:
                    nc.gpsimd.dma_start(qt[:, :512], qth[b][:, :512])
                    nc.gpsimd.dma_start(qt[:, 512:], qth[b][:, 512:])
                else:
                    nc.gpsimd.dma_start(qt, qth[b])

                # K^T on the SP queue. Only the very first seq is split
                # into two tiles (so its first matmuls start before the
                # full K^T lands).
                ksplit = KSPLIT if first else NTBMAX * 128
                ka_cols = min(ksplit, ntb * 128)
                kta = kt_pool.tile([128, ka_cols], bf16, tag="kta")
                nc.sync.dma_start(kta, kth[b][:, :ka_cols])
                if ntb * 128 > ksplit:
                    kb_cols = ntb * 128 - ksplit
                    ktb = kt_pool.tile([128, kb_cols], bf16, tag="ktb", name="ktb")
                    if first:
                        # halves: early t-blocks unblock before the full load
                        nc.sync.dma_start(
                            ktb[:, : kb_cols // 2],
                            kth[b][:, ksplit : ksplit + kb_cols // 2],
                        )
                        nc.sync.dma_start(
                            ktb[:, kb_cols // 2 :],
                            kth[b][:, ksplit + kb_cols // 2 : ntb * 128],
                        )
                    else:
                        nc.sync.dma_start(ktb, kth[b][:, ksplit : ntb * 128])
                else:
                    ktb = None

                # V natural [t%128, tb, d]; one contiguous DMA (first seq:
                # split so the first PV isn't gated on the full V load)
                vt = v_pool.tile([128, NTBMAX, HD], bf16, tag="vt")
                if first:
                    nc.gpsimd.dma_start(vt[:, :2, :], vth[b][:, :2, :])
                    nc.gpsimd.dma_start(vt[:, 2:ntb, :], vth[b][:, 2:ntb, :])
                    # masks are first needed by the exp at the ctx boundary
                    nc.gpsimd.dma_start(mask_all, mh)
                else:
                    nc.gpsimd.dma_start(vt[:, :ntb, :], vth[b][:, :ntb, :])

                load_tiles[b] = ((kta, ktb, ksplit), vt, qt)

            class Ctx:
                pass

            ctxs = {}

            def make_ctx(b, parity):
                sq = seqs[b]
                c = Ctx()
                c.b = b
                c.sq = sq
                c.kt, c.vt, c.qt = load_tiles.pop(b)
                c.mask = mask_all[:, b, :]
                c.ptt = pt_pool.tile([128, NTBMAX, SG], bf16, tag="pt", name="ptt")
                po, pd = (psum_a, psum_b) if parity == 0 else (psum_b, psum_a)
                c.outt = po.tile([128, SG], f32, tag="x", name="outt")
                c.dent = pd.tile([128, SG], f32, tag="x", name="dent")
                c.last_tb = [0, 0]
                for tb in range(sq["ntb"]):
                    for ci, (g0, g1) in enumerate(chunks):
                        if max(sq["qmin"][tb], g0) < g1:
                            c.last_tb[ci] = tb
                c.gi = 0
                c.acc = None
                c.pnext = 0
                c.nmm = [0, 0]
                c.nmm_tot = [
                    sum(1 for grp in sq["dgroups"] if max(sq["qmin"][grp[0]], g0) < g1)
                    for (g0, g1) in chunks
                ]
                c.mm_new = []
                c.mm_mid = []
                c.mm_ready = []
                c.done = -1
                c.finish_step = None
                c.closed = False
                return c

            def kt_sl(c, tb):
                kta, ktb, ksplit = c.kt
                if tb * 128 < ksplit:
                    return kta[:, tb * 128 : (tb + 1) * 128]
                return ktb[:, tb * 128 - ksplit : (tb + 1) * 128 - ksplit]

            def emit_scores(c, tb0):
                sq = c.sq
                qm = sq["qmin"][tb0]
                s_ps = psum_s.tile([128, SG], f32, tag="s", name="s_ps")
                for c0, c1 in ((qm * 128, 512), (max(512, qm * 128), SG)):
                    if c0 >= c1:
                        continue
                    nc.tensor.matmul(
                        s_ps[:, c0:c1],
                        lhsT=kt_sl(c, tb0),
                        rhs=c.qt[:, c0:c1],
                        start=True,
                        stop=True,
                    )
                nc.scalar.activation(
                    out=c.ptt[:, tb0, qm * 128 :],
                    in_=s_ps[:, qm * 128 : SG],
                    func=mybir.ActivationFunctionType.Exp,
                    scale=SCALE,
                    bias=(0.0 if sq["valid"][tb0] else c.mask[:, tb0 : tb0 + 1]),
                )
                # staircase: zero P^T rows of new-token blocks for
                # earlier q-tiles inside this t-block's suffix
                ctxp32, ttot = sq["ctxp32"], sq["ttot"]
                for r0 in range(0, 128, 32):
                    t0 = tb0 * 128 + r0
                    if t0 < ctxp32 or t0 >= ttot:
                        continue
                    blk = (t0 - ctxp32) // 32
                    if blk > qm:
                        nc.vector.memset(
                            c.ptt[r0 : r0 + 32, tb0, qm * 128 : blk * 128], 0.0
                        )

            def emit_pv(c, tb):
                qm = c.sq["qmin"][tb]
                for ci, (g0, g1) in enumerate(chunks):
                    lo = max(qm, g0)
                    if lo >= g1:
                        continue
                    nc.tensor.matmul(
                        c.outt[:, lo * 128 : g1 * 128],
                        lhsT=c.vt[:, tb, :],
                        rhs=c.ptt[:, tb, lo * 128 : g1 * 128],
                        start=(tb == 0),
                        stop=(tb == c.last_tb[ci]),
                    )

            def emit_dent_mms(c, only_chunk=None):
                # emit queued ones-matmuls (their DVE adds are long done)
                keep = []
                for ci, qm, rhs_of in c.mm_ready:
                    if only_chunk is not None and ci != only_chunk:
                        keep.append((ci, qm, rhs_of))
                        continue
                    g0, g1 = chunks[ci]
                    lo = max(qm, g0)
                    nc.tensor.matmul(
                        c.dent[:, lo * 128 : g1 * 128],
                        lhsT=ones_t,
                        rhs=rhs_of(lo * 128, g1 * 128),
                        start=(c.nmm[ci] == 0),
                        stop=(c.nmm[ci] + 1 == c.nmm_tot[ci]),
                    )
                    c.nmm[ci] += 1
                c.mm_ready = keep

            def stage_new(c, ready_upto):
                # stage newly-ready groups: incremental left-leaning DVE
                # accumulator (pair-add + running join per 2 t-blocks), so
                # each pipeline step carries at most ~2 adds
                sq = c.sq
                dgroups = sq["dgroups"]
                while c.gi < len(dgroups):
                    grp = dgroups[c.gi]
                    qm = sq["qmin"][grp[0]]
                    c0 = qm * 128
                    n = len(grp)
                    ptt = c.ptt
                    while c.pnext + 1 < n and grp[c.pnext + 1] <= ready_upto:
                        t = dsum_pool.tile([128, SG], bf16, tag="ds1", name="ds1")
                        nc.vector.tensor_add(
                            t[:, c0:],
                            ptt[:, grp[c.pnext], c0:],
                            ptt[:, grp[c.pnext + 1], c0:],
                        )
                        if c.acc is None:
                            c.acc = t
                        else:
                            t2 = dsum_pool.tile([128, SG], bf16, tag="ds2", name="ds2")
                            nc.vector.tensor_add(t2[:, c0:], c.acc[:, c0:], t[:, c0:])
                            c.acc = t2
                        c.pnext += 2
                    if grp[-1] > ready_upto:
                        break
                    if n == 1:
                        rhs_of = lambda a, b, ptt=ptt, tb=grp[0]: ptt[:, tb, a:b]
                    else:
                        if c.pnext < n:  # odd run: fold in the last t-block
                            t2 = dsum_pool.tile([128, SG], bf16, tag="ds2", name="ds2")
                            nc.vector.tensor_add(
                                t2[:, c0:], c.acc[:, c0:], ptt[:, grp[-1], c0:]
                            )
                            c.acc = t2
                        rhs_of = lambda a, b, ds=c.acc: ds[:, a:b]
                        c.acc = None
                        c.pnext = 0
                    for ci, (g0, g1) in enumerate(chunks):
                        if max(qm, g0) < g1:
                            c.mm_new.append((ci, qm, rhs_of))
                    c.gi += 1

            def emit_half(c, h):
                # normalize + store one sg-half: OUT^T * (1/denom) -> bf16
                h0, h1 = h * (SG // 2), (h + 1) * (SG // 2)
                invh = out_pool.tile([128, SG // 2], f32, tag="invh", name="invh")
                nc.vector.reciprocal_approx_fast(invh, c.dent[:, h0:h1])
                otfh = out_pool.tile([128, SG // 2], bf16, tag="otfh", name="otfh")
                nc.vector.tensor_mul(otfh, c.outt[:, h0:h1], invh)
                nc.sync.dma_start(outh[c.b][:, h0:h1], otfh)

            def close_seq(c):
                # per-chunk: half 0's reciprocal overlaps chunk 1's final
                # denominator matmuls on PE
                c.mm_ready += c.mm_mid + c.mm_new
                c.mm_mid = []
                c.mm_new = []
                emit_dent_mms(c, only_chunk=0)
                emit_half(c, 0)
                emit_dent_mms(c, only_chunk=1)
                emit_half(c, 1)
                c.closed = True

            def post(c, ptb, step):
                emit_pv(c, ptb)
                c.done = ptb
                stage_new(c, ptb)
                c.mm_ready += c.mm_new
                c.mm_new = []
                if ptb == c.sq["ntb"] - 1:
                    c.finish_step = step

            # ---- the flat t-block pipeline across all sequences ----
            stream = [(b, tb) for b in order for tb in range(seqs[b]["ntb"])]

            emit_loads(order[0], first=True)

            # pre-warm the PE clock (HAM) with dummy matmuls while the
            # first loads are in flight
            warm_rhs = const_pool.tile([128, 512], bf16)
            nc.vector.memset(warm_rhs, 0.0)
            warm_ps = psum_s.tile([128, SG], f32, tag="s", name="s_ps")
            for _ in range(8):
                nc.tensor.matmul(
                    warm_ps[:, :512], lhsT=ones_t, rhs=warm_rhs,
                    start=True, stop=True,
                )
            warm_sink = const_pool.tile([1, 1], f32)
            nc.vector.tensor_copy(warm_sink, warm_ps[0:1, 0:1])

            emit_loads(order[1])

            seq_idx = {b: j for j, b in enumerate(order)}
            nseq = 0
            for i, (b, tb) in enumerate(stream):
                if b not in ctxs:
                    ctxs[b] = make_ctx(b, nseq % 2)
                    nseq += 1
                    j = seq_idx[b]
                    if j + 2 < B:
                        emit_loads(order[j + 2])
                emit_scores(ctxs[b], tb)
                # close any sequence finished on an EARLIER step (its final
                # DVE adds ran last step). Must precede post(): the first PV
                # of this seq writes the closed seq's recycled PSUM banks.
                for pc in list(ctxs.values()):
                    if pc.finish_step is not None and pc.finish_step < i and not pc.closed:
                        close_seq(pc)
                # PV runs at lag 2: its exp finished ~2 steps ago, so it
                # never stalls the in-order PE queue (which would delay the
                # next scores and starve ACT)
                if i >= 2:
                    pb, ptb = stream[i - 2]
                    pc = ctxs[pb]
                    # denominator matmuls staged earlier: their DVE adds
                    # are long done, so they never stall the PE queue
                    emit_dent_mms(pc)
                    post(pc, ptb, i)
            # drain the pipeline: last two t-blocks' PVs + the final close
            n = len(stream)
            for j in (n - 2, n - 1):
                pb, ptb = stream[j]
                emit_dent_mms(ctxs[pb])
                post(ctxs[pb], ptb, n)
            for pc in ctxs.values():
                if pc.finish_step is not None and not pc.closed:
                    close_seq(pc)
    return nc


def _compile(seqs, order):
    import concourse.bacc as bacc

    nc = bacc.Bacc(
        "TRN2",
        target_bir_lowering=False,
        debug=False,
        enable_asserts=False,
        num_devices=8,
    )
    _build(nc, seqs, order)
    nc.compile()
    return nc


def kernel(q, k, v, k_cache, v_cache, page_tables, context_lens, page_size, block_size, **_):
    from concourse import bass_utils

    q = np.asarray(q)
    k = np.asarray(k)
    v = np.asarray(v)
    k_cache = np.asarray(k_cache)
    v_cache = np.asarray(v_cache)
    page_tables = np.asarray(page_tables)
    context_lens = np.asarray(context_lens)
    assert int(page_size) == PAGE and int(block_size) == BLOCK
    assert q.shape == (B * S, NUM_HEADS * HD)
    assert page_tables.shape == (B, MAX_PAGES)

    seqs, order = _schedule(context_lens)
    nc = _compile(seqs, order)

    bf = ml_dtypes.bfloat16
    masks = _masks(seqs)

    # host-side assembly: per-seq contiguous padded K/V in [t, n, d] layout
    TPAD = NTBMAX * 128
    kasm = np.zeros((B, TPAD, NUM_KV_HEADS, HD), bf)
    vasm = np.zeros((B, TPAD, NUM_KV_HEADS, HD), bf)
    kcv = k_cache.reshape(MAX_PAGES * B * PAGE, NUM_KV_HEADS, HD)
    vcv = v_cache.reshape(MAX_PAGES * B * PAGE, NUM_KV_HEADS, HD)
    kv = k.reshape(B * S, NUM_KV_HEADS, HD)
    vv = v.reshape(B * S, NUM_KV_HEADS, HD)
    for b, sq in enumerate(seqs):
        ctx, ctxp32 = sq["ctx"], sq["ctxp32"]
        npg = sq["ctxp"] // PAGE
        pages = page_tables[b, :npg].astype(np.int64)
        rows = (pages[:, None] * PAGE + np.arange(PAGE)[None, :]).reshape(-1)[:ctx]
        kasm[b, :ctx] = kcv[rows].astype(bf)
        vasm[b, :ctx] = vcv[rows].astype(bf)
        kasm[b, ctxp32 : ctxp32 + S] = kv[b * S : (b + 1) * S].astype(bf)
        vasm[b, ctxp32 : ctxp32 + S] = vv[b * S : (b + 1) * S].astype(bf)

    # device layouts: K^T [b, d, t]; V [b, t%128, tb, d]; Q^T [b, d, sg]
    kth_all = np.ascontiguousarray(kasm.transpose(2, 0, 3, 1))  # [n, B, HD, TPAD]
    vth_all = np.ascontiguousarray(
        vasm.reshape(B, NTBMAX, 128, NUM_KV_HEADS, HD).transpose(3, 0, 2, 1, 4)
    )  # [n, B, 128, NTBMAX, HD]
    qv = q.reshape(B, S, NUM_KV_HEADS, G, HD)
    qth_all = np.ascontiguousarray(
        qv.transpose(2, 0, 4, 1, 3).reshape(NUM_KV_HEADS, B, HD, SG)
    ).astype(bf)  # [n, B, HD, SG=(s,g)] ... via [n, B, d, s, g]

    in_maps = []
    for n in range(NUM_KV_HEADS):
        in_maps.append(
            {
                "kth": kth_all[n],
                "vth": vth_all[n],
                "qth": qth_all[n],
                "mh": masks,
            }
        )

    res = bass_utils.run_bass_kernel_spmd(nc, in_maps, core_ids=list(range(8)))
    global _last_results
    _last_results = res
    # per-core outh is [B, HD, SG=(s,g)] bf16; assemble [B*S, (n,g)*HD] f32
    out = np.empty((B * S, NUM_HEADS * HD), np.float32)
    ov = out.reshape(B, S, NUM_KV_HEADS, G, HD)
    for n in range(NUM_KV_HEADS):
        # [B, HD, S*G] -> [B, S, G, HD]
        on = res.results[n]["outh"].astype(np.float32).reshape(B, HD, S, G)
        ov[:, :, n, :, :] = on.transpose(0, 2, 3, 1)
    return out


_last_results = None


# revision 40
# speedup vs baseline: 1.2020x; 1.2020x over previous
# Paged sparse attention (GQA, block-masked new tokens) on 8 TRN2 NeuronCores.
#
# Sharding: tensor-parallel over the 8 KV heads (one KV head + its 4 Q heads
# per core). Every core sees all 8 sequences, so the compiled schedule
# (derived from page_tables/context_lens, identical across cores) is SPMD.
#
# Orientation: scores are computed TRANSPOSED (S^T[t, sg] per 128-row
# t-block, K^T-stationary, Q^T-moving), so the exp (ACT) writes P^T directly
# in the layout the PV matmul consumes — no probability transposes anywhere.
# Masking folds into the exp's per-partition bias (host-precomputed -1e30
# rows for the partial page / 32-alignment gap / tail pad). The softmax
# denominator comes from an extra matmul with an all-ones stationary matrix,
# which leaves the per-sg denominator replicated across all 128 PSUM
# partitions — the normalization is then a fused (OUT^T * 1/denom -> bf16)
# DVE pass, transposed back to [sg, d] by the host during the final gather.
#
# The host pre-assembles, per sequence, contiguous zero-padded K^T / V / Q^T
# buffers (page-table gather, 32-alignment gap, new tokens, tail pad all
# resolved in numpy), so every device load is one large contiguous DMA —
# no on-device DMA transposes, no gather runs, no memsets.
#
# The whole kernel is emitted as ONE flat software pipeline over t-blocks
# across all sequences: scores(i) issue ahead of PV(i-1), so at sequence
# boundaries the next sequence's first exp is never stuck behind the
# previous sequence's tail matmuls. outt/dent PSUM banks swap roles between
# consecutive sequences so the first PV of a sequence only waits on the
# previous sequence's (cheap) denominator-reciprocal read, not its full
# normalization.
#
# The block-causal mask for new tokens reduces (with sg = s*4+g ordering) to
# a suffix of valid sg columns per t-block (plus a small intra-block
# staircase zeroed on the bf16 P^T), so invalid regions are simply never
# computed. The softmax denominator streams P^T through a ones-matmul; runs
# of up to 16 qmin-equal t-blocks are pre-summed on DVE (incremental
# left-leaning bf16 accumulator, <=2 adds per pipeline step) so the
# ones-matmul touches each column once per run. Sequences are processed
# 2nd-largest first and LARGEST last: the final sequence's long context
# phase absorbs every other sequence's deferred denominator/normalize/store
# work, and its last group is a single t-block, keeping the endgame chain
# (exp -> matmul -> reciprocal -> mul -> store) short.

import sys

sys.path.insert(0, "/opt/trn_rl_repo")

import ml_dtypes
import numpy as np

B = 8
S = 256
NUM_HEADS = 32
NUM_KV_HEADS = 8
G = NUM_HEADS // NUM_KV_HEADS  # 4
HD = 128
PAGE = 16
BLOCK = 32
MAX_PAGES = 128
C = MAX_PAGES * PAGE  # 2048
SCALE = 0.08838834764831845
SG = S * G  # 1024 q rows per (seq, kv head)
TMAX = C + S + 32  # worst-case padded length
NTBMAX = (TMAX + 127) // 128
NQT = SG // 128  # 8 q-tiles per seq

NEG = -1e30


def _schedule(context_lens: np.ndarray):
    """Per-seq schedule baked into the compiled kernel (same on all cores)."""
    desc = sorted(
        range(B),
        key=lambda b: -(((int(context_lens[b]) + 15) // 16 * 16 + 31) // 32 * 32),
    )
    # 2nd-largest first (covers load latency), then descending, with the
    # LARGEST last: its long context phase absorbs every other sequence's
    # deferred denominator/close work, so almost nothing piles up after
    # the final exp
    order = desc[1:] + desc[:1]
    seqs = []
    for b in range(B):
        ctx = int(context_lens[b])
        npg = (ctx + PAGE - 1) // PAGE
        ctxp = npg * PAGE
        ctxp32 = ((ctxp + 31) // 32) * 32  # 32-align the new-token region
        ttot = ctxp32 + S
        ntb = (ttot + 127) // 128
        tq = [ctxp32 + BLOCK * (i + 1) for i in range(NQT)]
        # first valid q-tile per t-block (valid sg columns = suffix)
        qmin = [next(i for i in range(NQT) if tq[i] > tb * 128) for tb in range(ntb)]

        def fully_valid(tb):
            # every t-row in the block is a real, unmasked token
            if (tb + 1) * 128 > ttot:
                return False
            return not (ctx < (tb + 1) * 128 and tb * 128 < ctxp32)

        # denominator groups: runs of qmin-equal t-blocks pre-summed on DVE.
        # Last seq: single final group = short endgame chain.
        maxrun = 16
        dgroups = []
        tb = 0
        while tb < ntb:
            n = 1
            while n < maxrun and tb + n < ntb and qmin[tb + n] == qmin[tb]:
                n += 1
            if b == order[-1] and tb + n == ntb and n > 1:
                n -= 1  # force the final group to be a single
            dgroups.append(tuple(range(tb, tb + n)))
            tb += n
        seqs.append(
            dict(
                ctx=ctx,
                ctxp=ctxp,
                ctxp32=ctxp32,
                ttot=ttot,
                ntb=ntb,
                tq=tq,
                qmin=qmin,
                dgroups=dgroups,
                valid=[fully_valid(tb) for tb in range(ntb)],
            )
        )
    return seqs, order


def _masks(seqs):
    """Host-precomputed per-partition exp bias: [128, B, NTBMAX] fp32.
    mask[p, b, tb] is added (post-scale) to scores of t-row tb*128+p:
    0 for valid rows, -1e30 for masked rows (partial page, 32-align gap,
    padded tail)."""
    m = np.zeros((B, 128, NTBMAX), np.float32)
    for b, sq in enumerate(seqs):
        valid = np.zeros((NTBMAX * 128,), bool)
        valid[: sq["ttot"]] = True
        valid[sq["ctx"] : sq["ctxp32"]] = False  # partial page + gap
        m[b][~valid.reshape(NTBMAX, 128).T] = NEG
    return np.ascontiguousarray(m.transpose(1, 0, 2))


def _build(nc, seqs, order):
    import concourse.mybir as mybir
    import concourse.tile as tile

    bf16 = mybir.dt.bfloat16
    f32 = mybir.dt.float32

    # host-assembled, per-seq contiguous + padded, transposed layouts
    kth = nc.dram_tensor("kth", [B, HD, NTBMAX * 128], bf16, kind="ExternalInput").ap()
    vth = nc.dram_tensor("vth", [B, 128, NTBMAX, HD], bf16, kind="ExternalInput").ap()
    qth = nc.dram_tensor("qth", [B, HD, SG], bf16, kind="ExternalInput").ap()
    mh = nc.dram_tensor("mh", [128, B, NTBMAX], f32, kind="ExternalInput").ap()
    # transposed output [b, d, sg] in bf16; the host normz-reindexes
    outh = nc.dram_tensor("outh", [B, HD, SG], bf16, kind="ExternalOutput").ap()

    KSPLIT = 128  # first seq: tiny first K^T tile so matmuls start early
    chunks = ((0, 4), (4, 8))

    with tile.TileContext(nc) as tc:
        with (
            tc.tile_pool(name="cst", bufs=1) as const_pool,
            tc.tile_pool(name="kt", bufs=3) as kt_pool,
            tc.tile_pool(name="vt", bufs=3) as v_pool,
            tc.tile_pool(name="qt", bufs=3) as qt_pool,
            tc.tile_pool(name="pt", bufs=3) as pt_pool,
            tc.tile_pool(name="ds", bufs=6) as dsum_pool,
            tc.tile_pool(name="ot", bufs=2) as out_pool,
            tc.tile_pool(name="ps_s", bufs=2, space="PSUM") as psum_s,
            tc.tile_pool(name="ps_a", bufs=1, space="PSUM") as psum_a,
            tc.tile_pool(name="ps_b", bufs=1, space="PSUM") as psum_b,
        ):
            ones_t = const_pool.tile([128, 128], bf16)
            nc.vector.memset(ones_t, 1.0)

            mask_all = const_pool.tile([128, B, NTBMAX], f32)

            load_tiles = {}

            def emit_loads(b, first=False):
                sq = seqs[b]
                ntb = sq["ntb"]

                # Q^T first on the gpsimd queue: the first score matmul
                # needs only qt + the first K^T tile
                qt = qt_pool.tile([128, SG], bf16, tag="qt")
                if first:
                    nc.gpsimd.dma_start(qt[:, :512], qth[b][:, :512])
                    nc.gpsimd.dma_start(qt[:, 512:], qth[b][:, 512:])
                else:
                    nc.gpsimd.dma_start(qt, qth[b])

                # K^T on the SP queue. Only the very first seq is split
                # into two tiles (so its first matmuls start before the
                # full K^T lands).
                ksplit = KSPLIT if first else NTBMAX * 128
                ka_cols = min(ksplit, ntb * 128)
                kta = kt_pool.tile([128, ka_cols], bf16, tag="kta")
                nc.sync.dma_start(kta, kth[b][:, :ka_cols])
                if ntb * 128 > ksplit:
                    kb_cols = ntb * 128 - ksplit
                    ktb = kt_pool.tile([128, kb_cols], bf16, tag="ktb", name="ktb")
                    if first:
                        # halves: early t-blocks unblock before the full load
                        nc.sync.dma_start(
                            ktb[:, : kb_cols // 2],
                            kth[b][:, ksplit : ksplit + kb_cols // 2],
                        )
                        nc.sync.dma_start(
                            ktb[:, kb_cols // 2 :],
                            kth[b][:, ksplit + kb_cols // 2 : ntb * 128],
                        )
                    else:
                        nc.sync.dma_start(ktb, kth[b][:, ksplit : ntb * 128])
                else:
                    ktb = None

                # V natural [t%128, tb, d]; one contiguous DMA (first seq:
                # split so the first PV isn't gated on the full V load)
                vt = v_pool.tile([128, NTBMAX, HD], bf16, tag="vt")
                if first:
                    nc.gpsimd.dma_start(vt[:, :2, :], vth[b][:, :2, :])
                    nc.gpsimd.dma_start(vt[:, 2:ntb, :], vth[b][:, 2:ntb, :])
                    # masks are first needed by the exp at the ctx boundary
                    nc.gpsimd.dma_start(mask_all, mh)
                else:
                    nc.gpsimd.dma_start(vt[:, :ntb, :], vth[b][:, :ntb, :])

                load_tiles[b] = ((kta, ktb, ksplit), vt, qt)

            class Ctx:
                pass

            ctxs = {}

            def make_ctx(b, parity):
                sq = seqs[b]
                c = Ctx()
                c.b = b
                c.sq = sq
                c.kt, c.vt, c.qt = load_tiles.pop(b)
                c.mask = mask_all[:, b, :]
                c.ptt = pt_pool.tile([128, NTBMAX, SG], bf16, tag="pt", name="ptt")
                po, pd = (psum_a, psum_b) if parity == 0 else (psum_b, psum_a)
                c.outt = po.tile([128, SG], f32, tag="x", name="outt")
                c.dent = pd.tile([128, SG], f32, tag="x", name="dent")
                c.last_tb = [0, 0]
                for tb in range(sq["ntb"]):
                    for ci, (g0, g1) in enumerate(chunks):
                        if max(sq["qmin"][tb], g0) < g1:
                            c.last_tb[ci] = tb
                c.gi = 0
                c.acc = None
                c.pnext = 0
                c.nmm = [0, 0]
                c.nmm_tot = [
                    sum(1 for grp in sq["dgroups"] if max(sq["qmin"][grp[0]], g0) < g1)
                    for (g0, g1) in chunks
                ]
                c.mm_new = []
                c.mm_mid = []
                c.mm_ready = []
                c.done = -1
                c.finish_step = None
                c.closed = False
                return c

            def kt_sl(c, tb):
                kta, ktb, ksplit = c.kt
                if tb * 128 < ksplit:
                    return kta[:, tb * 128 : (tb + 1) * 128]
                return ktb[:, tb * 128 - ksplit : (tb + 1) * 128 - ksplit]

            def emit_scores(c, tb0):
                sq = c.sq
                qm = sq["qmin"][tb0]
                s_ps = psum_s.tile([128, SG], f32, tag="s", name="s_ps")
                for c0, c1 in ((qm * 128, 512), (max(512, qm * 128), SG)):
                    if c0 >= c1:
                        continue
                    nc.tensor.matmul(
                        s_ps[:, c0:c1],
                        lhsT=kt_sl(c, tb0),
                        rhs=c.qt[:, c0:c1],
                        start=True,
                        stop=True,
                    )
                nc.scalar.activation(
                    out=c.ptt[:, tb0, qm * 128 :],
                    in_=s_ps[:, qm * 128 : SG],
                    func=mybir.ActivationFunctionType.Exp,
                    scale=SCALE,
                    bias=(0.0 if sq["valid"][tb0] else c.mask[:, tb0 : tb0 + 1]),
                )
                # staircase: zero P^T rows of new-token blocks for
                # earlier q-tiles inside this t-block's suffix
                ctxp32, ttot = sq["ctxp32"], sq["ttot"]
                for r0 in range(0, 128, 32):
                    t0 = tb0 * 128 + r0
                    if t0 < ctxp32 or t0 >= ttot:
                        continue
                    blk = (t0 - ctxp32) // 32
                    if blk > qm:
                        nc.vector.memset(
                            c.ptt[r0 : r0 + 32, tb0, qm * 128 : blk * 128], 0.0
                        )

            def emit_pv(c, tb):
                qm = c.sq["qmin"][tb]
                for ci, (g0, g1) in enumerate(chunks):
                    lo = max(qm, g0)
                    if lo >= g1:
                        continue
                    nc.tensor.matmul(
                        c.outt[:, lo * 128 : g1 * 128],
                        lhsT=c.vt[:, tb, :],
                        rhs=c.ptt[:, tb, lo * 128 : g1 * 128],
                        start=(tb == 0),
                        stop=(tb == c.last_tb[ci]),
                    )

            def emit_dent_mms(c, only_chunk=None):
                # emit queued ones-matmuls (their DVE adds are long done)
                keep = []
                for ci, qm, rhs_of in c.mm_ready:
                    if only_chunk is not None and ci != only_chunk:
                        keep.append((ci, qm, rhs_of))
                        continue
                    g0, g1 = chunks[ci]
                    lo = max(qm, g0)
                    nc.tensor.matmul(
                        c.dent[:, lo * 128 : g1 * 128],
                        lhsT=ones_t,
                        rhs=rhs_of(lo * 128, g1 * 128),
                        start=(c.nmm[ci] == 0),
                        stop=(c.nmm[ci] + 1 == c.nmm_tot[ci]),
                    )
                    c.nmm[ci] += 1
                c.mm_ready = keep

            def stage_new(c, ready_upto):
                # stage newly-ready groups: incremental left-leaning DVE
                # accumulator (pair-add + running join per 2 t-blocks), so
                # each pipeline step carries at most ~2 adds
                sq = c.sq
                dgroups = sq["dgroups"]
                while c.gi < len(dgroups):
                    grp = dgroups[c.gi]
                    qm = sq["qmin"][grp[0]]
                    c0 = qm * 128
                    n = len(grp)
                    ptt = c.ptt
                    while c.pnext + 1 < n and grp[c.pnext + 1] <= ready_upto:
                        t = dsum_pool.tile([128, SG], bf16, tag="ds1", name="ds1")
                        nc.vector.tensor_add(
                            t[:, c0:],
                            ptt[:, grp[c.pnext], c0:],
                            ptt[:, grp[c.pnext + 1], c0:],
                        )
                        if c.acc is None:
                            c.acc = t
                        else:
                            t2 = dsum_pool.tile([128, SG], bf16, tag="ds2", name="ds2")
                            nc.vector.tensor_add(t2[:, c0:], c.acc[:, c0:], t[:, c0:])
                            c.acc = t2
                        c.pnext += 2
                    if grp[-1] > ready_upto:
                        break
                    if n == 1:
                        rhs_of = lambda a, b, ptt=ptt, tb=grp[0]: ptt[:, tb, a:b]
                    else:
                        if c.pnext < n:  # odd run: fold in the last t-block
                            t2 = dsum_pool.tile([128, SG], bf16, tag="ds2", name="ds2")
                            nc.vector.tensor_add(
                                t2[:, c0:], c.acc[:, c0:], ptt[:, grp[-1], c0:]
                            )
                            c.acc = t2
                        rhs_of = lambda a, b, ds=c.acc: ds[:, a:b]
                        c.acc = None
                        c.pnext = 0
                    for ci, (g0, g1) in enumerate(chunks):
                        if max(qm, g0) < g1:
                            c.mm_new.append((ci, qm, rhs_of))
                    c.gi += 1

            pending_stores = []

            def emit_half(c, h, defer=True):
                # normalize one sg-half: OUT^T * (1/denom) -> bf16. The
                # store DMA is deferred one step so its wait on the mul is
                # pre-satisfied and never head-of-line-blocks the SP queue
                # (which would delay the next K^T load issue behind it).
                h0, h1 = h * (SG // 2), (h + 1) * (SG // 2)
                invh = out_pool.tile([128, SG // 2], f32, tag="invh", name="invh")
                nc.vector.reciprocal_approx_fast(invh, c.dent[:, h0:h1])
                otfh = out_pool.tile([128, SG // 2], bf16, tag="otfh", name="otfh")
                nc.vector.tensor_mul(otfh, c.outt[:, h0:h1], invh)
                if defer:
                    pending_stores.append((outh[c.b][:, h0:h1], otfh))
                else:
                    nc.sync.dma_start(outh[c.b][:, h0:h1], otfh)

            def flush_stores():
                while pending_stores:
                    dst, src = pending_stores.pop(0)
                    nc.sync.dma_start(dst, src)

            def close_seq(c):
                # per-chunk: half 0's reciprocal overlaps chunk 1's final
                # denominator matmuls on PE
                c.mm_ready += c.mm_mid + c.mm_new
                c.mm_mid = []
                c.mm_new = []
                defer = c.b != order[-1]  # last seq: store immediately
                emit_dent_mms(c, only_chunk=0)
                emit_half(c, 0, defer)
                emit_dent_mms(c, only_chunk=1)
                emit_half(c, 1, defer)
                c.closed = True

            def post(c, ptb, step):
                emit_pv(c, ptb)
                c.done = ptb
                stage_new(c, ptb)
                c.mm_ready += c.mm_new
                c.mm_new = []
                if ptb == c.sq["ntb"] - 1:
                    c.finish_step = step

            # ---- the flat t-block pipeline across all sequences ----
            stream = [(b, tb) for b in order for tb in range(seqs[b]["ntb"])]

            emit_loads(order[0], first=True)

            # pre-warm the PE clock (HAM) with dummy matmuls while the
            # first loads are in flight
            warm_rhs = const_pool.tile([128, 512], bf16)
            nc.vector.memset(warm_rhs, 0.0)
            warm_ps = psum_s.tile([128, SG], f32, tag="s", name="s_ps")
            for _ in range(8):
                nc.tensor.matmul(
                    warm_ps[:, :512], lhsT=ones_t, rhs=warm_rhs,
                    start=True, stop=True,
                )
            warm_sink = const_pool.tile([1, 1], f32)
            nc.vector.tensor_copy(warm_sink, warm_ps[0:1, 0:1])

            emit_loads(order[1])

            seq_idx = {b: j for j, b in enumerate(order)}
            nseq = 0
            for i, (b, tb) in enumerate(stream):
                if b not in ctxs:
                    ctxs[b] = make_ctx(b, nseq % 2)
                    nseq += 1
                    j = seq_idx[b]
                    if j + 2 < B:
                        emit_loads(order[j + 2])
                emit_scores(ctxs[b], tb)
                # close any sequence finished on an EARLIER step (its final
                # DVE adds ran last step). Must precede post(): the first PV
                # of this seq writes the closed seq's recycled PSUM banks.
                for pc in list(ctxs.values()):
                    if pc.finish_step is not None and pc.finish_step < i and not pc.closed:
                        close_seq(pc)
                # PV runs at lag 2: its exp finished ~2 steps ago, so it
                # never stalls the in-order PE queue (which would delay the
                # next scores and starve ACT)
                if i >= 2:
                    pb, ptb = stream[i - 2]
                    pc = ctxs[pb]
                    # denominator matmuls staged earlier: their DVE adds
                    # are long done, so they never stall the PE queue
                    emit_dent_mms(pc)
                    post(pc, ptb, i)
                flush_stores()
            # drain the pipeline: last two t-blocks' PVs + the final close
            n = len(stream)
            for j in (n - 2, n - 1):
                pb, ptb = stream[j]
                emit_dent_mms(ctxs[pb])
                post(ctxs[pb], ptb, n)
            for pc in ctxs.values():
                if pc.finish_step is not None and not pc.closed:
                    close_seq(pc)
    return nc


def _compile(seqs, order):
    import concourse.bacc as bacc

    nc = bacc.Bacc(
        "TRN2",
        target_bir_lowering=False,
        debug=False,
        enable_asserts=False,
        num_devices=8,
    )
    _build(nc, seqs, order)
    nc.compile()
    return nc


def kernel(q, k, v, k_cache, v_cache, page_tables, context_lens, page_size, block_size, **_):
    from concourse import bass_utils

    q = np.asarray(q)
    k = np.asarray(k)
    v = np.asarray(v)
    k_cache = np.asarray(k_cache)
    v_cache = np.asarray(v_cache)
    page_tables = np.asarray(page_tables)
    context_lens = np.asarray(context_lens)
    assert int(page_size) == PAGE and int(block_size) == BLOCK
    assert q.shape == (B * S, NUM_HEADS * HD)
    assert page_tables.shape == (B, MAX_PAGES)

    seqs, order = _schedule(context_lens)
    nc = _compile(seqs, order)

    bf = ml_dtypes.bfloat16
    masks = _masks(seqs)

    # host-side assembly: per-seq contiguous padded K/V in [t, n, d] layout
    TPAD = NTBMAX * 128
    kasm = np.zeros((B, TPAD, NUM_KV_HEADS, HD), bf)
    vasm = np.zeros((B, TPAD, NUM_KV_HEADS, HD), bf)
    kcv = k_cache.reshape(MAX_PAGES * B * PAGE, NUM_KV_HEADS, HD)
    vcv = v_cache.reshape(MAX_PAGES * B * PAGE, NUM_KV_HEADS, HD)
    kv = k.reshape(B * S, NUM_KV_HEADS, HD)
    vv = v.reshape(B * S, NUM_KV_HEADS, HD)
    for b, sq in enumerate(seqs):
        ctx, ctxp32 = sq["ctx"], sq["ctxp32"]
        npg = sq["ctxp"] // PAGE
        pages = page_tables[b, :npg].astype(np.int64)
        rows = (pages[:, None] * PAGE + np.arange(PAGE)[None, :]).reshape(-1)[:ctx]
        kasm[b, :ctx] = kcv[rows].astype(bf)
        vasm[b, :ctx] = vcv[rows].astype(bf)
        kasm[b, ctxp32 : ctxp32 + S] = kv[b * S : (b + 1) * S].astype(bf)
        vasm[b, ctxp32 : ctxp32 + S] = vv[b * S : (b + 1) * S].astype(bf)

    # device layouts: K^T [b, d, t]; V [b, t%128, tb, d]; Q^T [b, d, sg]
    kth_all = np.ascontiguousarray(kasm.transpose(2, 0, 3, 1))  # [n, B, HD, TPAD]
    vth_all = np.ascontiguousarray(
        vasm.reshape(B, NTBMAX, 128, NUM_KV_HEADS, HD).transpose(3, 0, 2, 1, 4)
    )  # [n, B, 128, NTBMAX, HD]
    qv = q.reshape(B, S, NUM_KV_HEADS, G, HD)
    qth_all = np.ascontiguousarray(
        qv.transpose(2, 0, 4, 1, 3).reshape(NUM_KV_HEADS, B, HD, SG)
    ).astype(bf)  # [n, B, HD, SG=(s,g)] ... via [n, B, d, s, g]

    in_maps = []
    for n in range(NUM_KV_HEADS):
        in_maps.append(
            {
                "kth": kth_all[n],
                "vth": vth_all[n],
                "qth": qth_all[n],
                "mh": masks,
            }
        )

    res = bass_utils.run_bass_kernel_spmd(nc, in_maps, core_ids=list(range(8)))
    global _last_results
    _last_results = res
    # per-core outh is [B, HD, SG=(s,g)] bf16; assemble [B*S, (n,g)*HD] f32
    out = np.empty((B * S, NUM_HEADS * HD), np.float32)
    ov = out.reshape(B, S, NUM_KV_HEADS, G, HD)
    for n in range(NUM_KV_HEADS):
        # [B, HD, S*G] -> [B, S, G, HD]
        on = res.results[n]["outh"].astype(np.float32).reshape(B, HD, S, G)
        ov[:, :, n, :, :] = on.transpose(0, 2, 3, 1)
    return out


_last_results = None


# revision 41
# speedup vs baseline: 1.2345x; 1.0270x over previous
# Paged sparse attention (GQA, block-masked new tokens) on 8 TRN2 NeuronCores.
#
# Sharding: tensor-parallel over the 8 KV heads (one KV head + its 4 Q heads
# per core). Every core sees all 8 sequences, so the compiled schedule
# (derived from page_tables/context_lens, identical across cores) is SPMD.
#
# Orientation: scores are computed TRANSPOSED (S^T[t, sg] per 128-row
# t-block, K^T-stationary, Q^T-moving), so the exp (ACT) writes P^T directly
# in the layout the PV matmul consumes — no probability transposes anywhere.
# Masking folds into the exp's per-partition bias (host-precomputed -1e30
# rows for the partial page / 32-alignment gap / tail pad). The softmax
# denominator comes from an extra matmul with an all-ones stationary matrix,
# which leaves the per-sg denominator replicated across all 128 PSUM
# partitions — the normalization is then a fused (OUT^T * 1/denom -> bf16)
# DVE pass, transposed back to [sg, d] by the host during the final gather.
#
# The host pre-assembles, per sequence, contiguous zero-padded K^T / V / Q^T
# buffers (page-table gather, 32-alignment gap, new tokens, tail pad all
# resolved in numpy), so every device load is one large contiguous DMA —
# no on-device DMA transposes, no gather runs, no memsets.
#
# The whole kernel is emitted as ONE flat software pipeline over t-blocks
# across all sequences: scores(i) issue ahead of PV(i-1), so at sequence
# boundaries the next sequence's first exp is never stuck behind the
# previous sequence's tail matmuls. outt/dent PSUM banks swap roles between
# consecutive sequences so the first PV of a sequence only waits on the
# previous sequence's (cheap) denominator-reciprocal read, not its full
# normalization.
#
# The block-causal mask for new tokens reduces (with sg = s*4+g ordering) to
# a suffix of valid sg columns per t-block (plus a small intra-block
# staircase zeroed on the bf16 P^T), so invalid regions are simply never
# computed. The softmax denominator streams P^T through a ones-matmul; runs
# of up to 16 qmin-equal t-blocks are pre-summed on DVE (incremental
# left-leaning bf16 accumulator, <=2 adds per pipeline step) so the
# ones-matmul touches each column once per run. Sequences are processed
# 2nd-largest first and LARGEST last: the final sequence's long context
# phase absorbs every other sequence's deferred denominator/normalize/store
# work, and its last group is a single t-block, keeping the endgame chain
# (exp -> matmul -> reciprocal -> mul -> store) short.

import sys

sys.path.insert(0, "/opt/trn_rl_repo")

import ml_dtypes
import numpy as np

B = 8
S = 256
NUM_HEADS = 32
NUM_KV_HEADS = 8
G = NUM_HEADS // NUM_KV_HEADS  # 4
HD = 128
PAGE = 16
BLOCK = 32
MAX_PAGES = 128
C = MAX_PAGES * PAGE  # 2048
SCALE = 0.08838834764831845
SG = S * G  # 1024 q rows per (seq, kv head)
TMAX = C + S + 32  # worst-case padded length
NTBMAX = (TMAX + 127) // 128
NQT = SG // 128  # 8 q-tiles per seq

NEG = -1e30


def _schedule(context_lens: np.ndarray):
    """Per-seq schedule baked into the compiled kernel (same on all cores)."""
    desc = sorted(
        range(B),
        key=lambda b: -(((int(context_lens[b]) + 15) // 16 * 16 + 31) // 32 * 32),
    )
    # 2nd-largest first (covers load latency), then descending, with the
    # LARGEST last: its long context phase absorbs every other sequence's
    # deferred denominator/close work, so almost nothing piles up after
    # the final exp
    order = desc[1:] + desc[:1]
    seqs = []
    for b in range(B):
        ctx = int(context_lens[b])
        npg = (ctx + PAGE - 1) // PAGE
        ctxp = npg * PAGE
        ctxp32 = ((ctxp + 31) // 32) * 32  # 32-align the new-token region
        ttot = ctxp32 + S
        ntb = (ttot + 127) // 128
        tq = [ctxp32 + BLOCK * (i + 1) for i in range(NQT)]
        # first valid q-tile per t-block (valid sg columns = suffix)
        qmin = [next(i for i in range(NQT) if tq[i] > tb * 128) for tb in range(ntb)]

        def fully_valid(tb):
            # every t-row in the block is a real, unmasked token
            if (tb + 1) * 128 > ttot:
                return False
            return not (ctx < (tb + 1) * 128 and tb * 128 < ctxp32)

        # denominator groups: runs of qmin-equal t-blocks pre-summed on DVE.
        # Last seq: single final group = short endgame chain.
        maxrun = 16
        dgroups = []
        tb = 0
        while tb < ntb:
            n = 1
            while n < maxrun and tb + n < ntb and qmin[tb + n] == qmin[tb]:
                n += 1
            if b == order[-1] and tb + n == ntb and n > 1:
                n -= 1  # force the final group to be a single
            dgroups.append(tuple(range(tb, tb + n)))
            tb += n
        seqs.append(
            dict(
                ctx=ctx,
                ctxp=ctxp,
                ctxp32=ctxp32,
                ttot=ttot,
                ntb=ntb,
                tq=tq,
                qmin=qmin,
                dgroups=dgroups,
                valid=[fully_valid(tb) for tb in range(ntb)],
            )
        )
    return seqs, order


def _masks(seqs):
    """Host-precomputed per-partition exp bias: [128, B, NTBMAX] fp32.
    mask[p, b, tb] is added (post-scale) to scores of t-row tb*128+p:
    0 for valid rows, -1e30 for masked rows (partial page, 32-align gap,
    padded tail)."""
    m = np.zeros((B, 128, NTBMAX), np.float32)
    for b, sq in enumerate(seqs):
        valid = np.zeros((NTBMAX * 128,), bool)
        valid[: sq["ttot"]] = True
        valid[sq["ctx"] : sq["ctxp32"]] = False  # partial page + gap
        m[b][~valid.reshape(NTBMAX, 128).T] = NEG
    return np.ascontiguousarray(m.transpose(1, 0, 2))


def _build(nc, seqs, order):
    import concourse.mybir as mybir
    import concourse.tile as tile

    bf16 = mybir.dt.bfloat16
    f32 = mybir.dt.float32

    # host-assembled, per-seq contiguous + padded, transposed layouts
    kth = nc.dram_tensor("kth", [B, HD, NTBMAX * 128], bf16, kind="ExternalInput").ap()
    vth = nc.dram_tensor("vth", [B, 128, NTBMAX, HD], bf16, kind="ExternalInput").ap()
    qth = nc.dram_tensor("qth", [B, HD, SG], bf16, kind="ExternalInput").ap()
    mh = nc.dram_tensor("mh", [128, B, NTBMAX], f32, kind="ExternalInput").ap()
    # transposed output [b, d, sg] in bf16; the host normz-reindexes
    outh = nc.dram_tensor("outh", [B, HD, SG], bf16, kind="ExternalOutput").ap()

    KSPLIT = 128  # first seq: tiny first K^T tile so matmuls start early
    chunks = ((0, 4), (4, 8))

    with tile.TileContext(nc) as tc:
        with (
            tc.tile_pool(name="cst", bufs=1) as const_pool,
            tc.tile_pool(name="kt", bufs=3) as kt_pool,
            tc.tile_pool(name="vt", bufs=3) as v_pool,
            tc.tile_pool(name="qt", bufs=3) as qt_pool,
            tc.tile_pool(name="pt", bufs=3) as pt_pool,
            tc.tile_pool(name="ds", bufs=6) as dsum_pool,
            tc.tile_pool(name="ot", bufs=2) as out_pool,
            tc.tile_pool(name="ps_s", bufs=2, space="PSUM") as psum_s,
            tc.tile_pool(name="ps_a", bufs=1, space="PSUM") as psum_a,
            tc.tile_pool(name="ps_b", bufs=1, space="PSUM") as psum_b,
        ):
            ones_t = const_pool.tile([128, 128], bf16)
            nc.vector.memset(ones_t, 1.0)

            mask_all = const_pool.tile([128, B, NTBMAX], f32)

            load_tiles = {}

            def emit_loads(b, first=False):
                sq = seqs[b]
                ntb = sq["ntb"]

                # Q^T first on the gpsimd queue: the first score matmul
                # needs only qt + the first K^T tile
                qt = qt_pool.tile([128, SG], bf16, tag="qt")
                if first:
                    nc.gpsimd.dma_start(qt[:, :512], qth[b][:, :512])
                    nc.gpsimd.dma_start(qt[:, 512:], qth[b][:, 512:])
                else:
                    nc.gpsimd.dma_start(qt, qth[b])

                # K^T on the SP queue. Only the very first seq is split
                # into two tiles (so its first matmuls start before the
                # full K^T lands).
                ksplit = KSPLIT if first else NTBMAX * 128
                ka_cols = min(ksplit, ntb * 128)
                kta = kt_pool.tile([128, ka_cols], bf16, tag="kta")
                nc.sync.dma_start(kta, kth[b][:, :ka_cols])
                if ntb * 128 > ksplit:
                    kb_cols = ntb * 128 - ksplit
                    ktb = kt_pool.tile([128, kb_cols], bf16, tag="ktb", name="ktb")
                    if first:
                        # halves: early t-blocks unblock before the full load
                        nc.sync.dma_start(
                            ktb[:, : kb_cols // 2],
                            kth[b][:, ksplit : ksplit + kb_cols // 2],
                        )
                        nc.sync.dma_start(
                            ktb[:, kb_cols // 2 :],
                            kth[b][:, ksplit + kb_cols // 2 : ntb * 128],
                        )
                    else:
                        nc.sync.dma_start(ktb, kth[b][:, ksplit : ntb * 128])
                else:
                    ktb = None

                # V natural [t%128, tb, d]; one contiguous DMA (first seq:
                # split so the first PV isn't gated on the full V load)
                vt = v_pool.tile([128, NTBMAX, HD], bf16, tag="vt")
                if first:
                    nc.gpsimd.dma_start(vt[:, :2, :], vth[b][:, :2, :])
                    nc.gpsimd.dma_start(vt[:, 2:ntb, :], vth[b][:, 2:ntb, :])
                    # masks are first needed by the exp at the ctx boundary
                    nc.gpsimd.dma_start(mask_all, mh)
                else:
                    nc.gpsimd.dma_start(vt[:, :ntb, :], vth[b][:, :ntb, :])

                load_tiles[b] = ((kta, ktb, ksplit), vt, qt)

            class Ctx:
                pass

            ctxs = {}

            def make_ctx(b, parity):
                sq = seqs[b]
                c = Ctx()
                c.b = b
                c.sq = sq
                c.kt, c.vt, c.qt = load_tiles.pop(b)
                c.mask = mask_all[:, b, :]
                c.ptt = pt_pool.tile([128, NTBMAX, SG], bf16, tag="pt", name="ptt")
                po, pd = (psum_a, psum_b) if parity == 0 else (psum_b, psum_a)
                c.outt = po.tile([128, SG], f32, tag="x", name="outt")
                c.dent = pd.tile([128, SG], f32, tag="x", name="dent")
                c.last_tb = [0, 0]
                for tb in range(sq["ntb"]):
                    for ci, (g0, g1) in enumerate(chunks):
                        if max(sq["qmin"][tb], g0) < g1:
                            c.last_tb[ci] = tb
                c.gi = 0
                c.acc = None
                c.pnext = 0
                c.nmm = [0, 0]
                c.nmm_tot = [
                    sum(1 for grp in sq["dgroups"] if max(sq["qmin"][grp[0]], g0) < g1)
                    for (g0, g1) in chunks
                ]
                c.mm_new = []
                c.mm_mid = []
                c.mm_ready = []
                c.done = -1
                c.finish_step = None
                c.closed = False
                return c

            def kt_sl(c, tb):
                kta, ktb, ksplit = c.kt
                if tb * 128 < ksplit:
                    return kta[:, tb * 128 : (tb + 1) * 128]
                return ktb[:, tb * 128 - ksplit : (tb + 1) * 128 - ksplit]

            def emit_scores(c, tb0):
                sq = c.sq
                qm = sq["qmin"][tb0]
                s_ps = psum_s.tile([128, SG], f32, tag="s", name="s_ps")
                for c0, c1 in ((qm * 128, 512), (max(512, qm * 128), SG)):
                    if c0 >= c1:
                        continue
                    nc.tensor.matmul(
                        s_ps[:, c0:c1],
                        lhsT=kt_sl(c, tb0),
                        rhs=c.qt[:, c0:c1],
                        start=True,
                        stop=True,
                    )
                nc.scalar.activation(
                    out=c.ptt[:, tb0, qm * 128 :],
                    in_=s_ps[:, qm * 128 : SG],
                    func=mybir.ActivationFunctionType.Exp,
                    scale=SCALE,
                    bias=(0.0 if sq["valid"][tb0] else c.mask[:, tb0 : tb0 + 1]),
                )
                # staircase: zero P^T rows of new-token blocks for
                # earlier q-tiles inside this t-block's suffix
                ctxp32, ttot = sq["ctxp32"], sq["ttot"]
                for r0 in range(0, 128, 32):
                    t0 = tb0 * 128 + r0
                    if t0 < ctxp32 or t0 >= ttot:
                        continue
                    blk = (t0 - ctxp32) // 32
                    if blk > qm:
                        nc.vector.memset(
                            c.ptt[r0 : r0 + 32, tb0, qm * 128 : blk * 128], 0.0
                        )

            def emit_pv(c, tb):
                qm = c.sq["qmin"][tb]
                for ci, (g0, g1) in enumerate(chunks):
                    lo = max(qm, g0)
                    if lo >= g1:
                        continue
                    nc.tensor.matmul(
                        c.outt[:, lo * 128 : g1 * 128],
                        lhsT=c.vt[:, tb, :],
                        rhs=c.ptt[:, tb, lo * 128 : g1 * 128],
                        start=(tb == 0),
                        stop=(tb == c.last_tb[ci]),
                    )

            def emit_dent_mms(c, only_chunk=None):
                # emit queued ones-matmuls (their DVE adds are long done)
                keep = []
                for ci, qm, rhs_of in c.mm_ready:
                    if only_chunk is not None and ci != only_chunk:
                        keep.append((ci, qm, rhs_of))
                        continue
                    g0, g1 = chunks[ci]
                    lo = max(qm, g0)
                    nc.tensor.matmul(
                        c.dent[:, lo * 128 : g1 * 128],
                        lhsT=ones_t,
                        rhs=rhs_of(lo * 128, g1 * 128),
                        start=(c.nmm[ci] == 0),
                        stop=(c.nmm[ci] + 1 == c.nmm_tot[ci]),
                    )
                    c.nmm[ci] += 1
                c.mm_ready = keep

            def stage_new(c, ready_upto):
                # stage newly-ready groups: incremental left-leaning DVE
                # accumulator (pair-add + running join per 2 t-blocks), so
                # each pipeline step carries at most ~2 adds
                sq = c.sq
                dgroups = sq["dgroups"]
                while c.gi < len(dgroups):
                    grp = dgroups[c.gi]
                    qm = sq["qmin"][grp[0]]
                    c0 = qm * 128
                    n = len(grp)
                    ptt = c.ptt
                    while c.pnext + 1 < n and grp[c.pnext + 1] <= ready_upto:
                        t = dsum_pool.tile([128, SG], bf16, tag="ds1", name="ds1")
                        nc.vector.tensor_add(
                            t[:, c0:],
                            ptt[:, grp[c.pnext], c0:],
                            ptt[:, grp[c.pnext + 1], c0:],
                        )
                        if c.acc is None:
                            c.acc = t
                        else:
                            t2 = dsum_pool.tile([128, SG], bf16, tag="ds2", name="ds2")
                            nc.vector.tensor_add(t2[:, c0:], c.acc[:, c0:], t[:, c0:])
                            c.acc = t2
                        c.pnext += 2
                    if grp[-1] > ready_upto:
                        break
                    if n == 1:
                        rhs_of = lambda a, b, ptt=ptt, tb=grp[0]: ptt[:, tb, a:b]
                    else:
                        if c.pnext < n:  # odd run: fold in the last t-block
                            t2 = dsum_pool.tile([128, SG], bf16, tag="ds2", name="ds2")
                            nc.vector.tensor_add(
                                t2[:, c0:], c.acc[:, c0:], ptt[:, grp[-1], c0:]
                            )
                            c.acc = t2
                        rhs_of = lambda a, b, ds=c.acc: ds[:, a:b]
                        c.acc = None
                        c.pnext = 0
                    for ci, (g0, g1) in enumerate(chunks):
                        if max(qm, g0) < g1:
                            c.mm_new.append((ci, qm, rhs_of))
                    c.gi += 1

            pending_stores = []

            def emit_half(c, h, defer=True):
                # normalize one sg-half: OUT^T * (1/denom) -> bf16. The
                # store DMA is deferred one step so its wait on the mul is
                # pre-satisfied and never head-of-line-blocks the SP queue
                # (which would delay the next K^T load issue behind it).
                h0, h1 = h * (SG // 2), (h + 1) * (SG // 2)
                invh = out_pool.tile([128, SG // 2], f32, tag="invh", name="invh")
                nc.vector.reciprocal_approx_fast(invh, c.dent[:, h0:h1])
                otfh = out_pool.tile([128, SG // 2], bf16, tag="otfh", name="otfh")
                nc.vector.tensor_mul(otfh, c.outt[:, h0:h1], invh)
                if defer:
                    pending_stores.append((outh[c.b][:, h0:h1], otfh))
                else:
                    nc.sync.dma_start(outh[c.b][:, h0:h1], otfh)

            def flush_stores():
                while pending_stores:
                    dst, src = pending_stores.pop(0)
                    nc.sync.dma_start(dst, src)

            def close_seq(c):
                # per-chunk: half 0's reciprocal overlaps chunk 1's final
                # denominator matmuls on PE
                c.mm_ready += c.mm_mid + c.mm_new
                c.mm_mid = []
                c.mm_new = []
                defer = c.b != order[-1]  # last seq: store immediately
                emit_dent_mms(c, only_chunk=0)
                emit_half(c, 0, defer)
                emit_dent_mms(c, only_chunk=1)
                emit_half(c, 1, defer)
                c.closed = True

            def post(c, ptb, step):
                emit_pv(c, ptb)
                c.done = ptb
                stage_new(c, ptb)
                c.mm_ready += c.mm_mid
                c.mm_mid = c.mm_new
                c.mm_new = []
                if ptb == c.sq["ntb"] - 1:
                    c.finish_step = step

            # ---- the flat t-block pipeline across all sequences ----
            stream = [(b, tb) for b in order for tb in range(seqs[b]["ntb"])]

            emit_loads(order[0], first=True)

            # pre-warm the PE clock (HAM) with dummy matmuls while the
            # first loads are in flight
            warm_rhs = const_pool.tile([128, 512], bf16)
            nc.vector.memset(warm_rhs, 0.0)
            warm_ps = psum_s.tile([128, SG], f32, tag="s", name="s_ps")
            for _ in range(8):
                nc.tensor.matmul(
                    warm_ps[:, :512], lhsT=ones_t, rhs=warm_rhs,
                    start=True, stop=True,
                )
            warm_sink = const_pool.tile([1, 1], f32)
            nc.vector.tensor_copy(warm_sink, warm_ps[0:1, 0:1])

            emit_loads(order[1])

            seq_idx = {b: j for j, b in enumerate(order)}
            nseq = 0
            for i, (b, tb) in enumerate(stream):
                if b not in ctxs:
                    ctxs[b] = make_ctx(b, nseq % 2)
                    nseq += 1
                    j = seq_idx[b]
                    if j + 2 < B:
                        emit_loads(order[j + 2])
                emit_scores(ctxs[b], tb)
                # close any sequence finished on an EARLIER step (its final
                # DVE adds ran last step). Must precede post(): the first PV
                # of this seq writes the closed seq's recycled PSUM banks.
                for pc in list(ctxs.values()):
                    if pc.finish_step is not None and pc.finish_step < i and not pc.closed:
                        close_seq(pc)
                # PV runs at lag 2: its exp finished ~2 steps ago, so it
                # never stalls the in-order PE queue (which would delay the
                # next scores and starve ACT)
                if i >= 2:
                    pb, ptb = stream[i - 2]
                    pc = ctxs[pb]
                    # denominator matmuls staged earlier: their DVE adds
                    # are long done, so they never stall the PE queue
                    emit_dent_mms(pc)
                    post(pc, ptb, i)
                flush_stores()
            # drain the pipeline: last two t-blocks' PVs + the final close
            n = len(stream)
            for j in (n - 2, n - 1):
                pb, ptb = stream[j]
                emit_dent_mms(ctxs[pb])
                post(ctxs[pb], ptb, n)
            for pc in ctxs.values():
                if pc.finish_step is not None and not pc.closed:
                    close_seq(pc)
    return nc


def _compile(seqs, order):
    import concourse.bacc as bacc

    nc = bacc.Bacc(
        "TRN2",
        target_bir_lowering=False,
        debug=False,
        enable_asserts=False,
        num_devices=8,
    )
    _build(nc, seqs, order)
    nc.compile()
    return nc


def kernel(q, k, v, k_cache, v_cache, page_tables, context_lens, page_size, block_size, **_):
    from concourse import bass_utils

    q = np.asarray(q)
    k = np.asarray(k)
    v = np.asarray(v)
    k_cache = np.asarray(k_cache)
    v_cache = np.asarray(v_cache)
    page_tables = np.asarray(page_tables)
    context_lens = np.asarray(context_lens)
    assert int(page_size) == PAGE and int(block_size) == BLOCK
    assert q.shape == (B * S, NUM_HEADS * HD)
    assert page_tables.shape == (B, MAX_PAGES)

    seqs, order = _schedule(context_lens)
    nc = _compile(seqs, order)

    bf = ml_dtypes.bfloat16
    masks = _masks(seqs)

    # host-side assembly: per-seq contiguous padded K/V in [t, n, d] layout
    TPAD = NTBMAX * 128
    kasm = np.zeros((B, TPAD, NUM_KV_HEADS, HD), bf)
    vasm = np.zeros((B, TPAD, NUM_KV_HEADS, HD), bf)
    kcv = k_cache.reshape(MAX_PAGES * B * PAGE, NUM_KV_HEADS, HD)
    vcv = v_cache.reshape(MAX_PAGES * B * PAGE, NUM_KV_HEADS, HD)
    kv = k.reshape(B * S, NUM_KV_HEADS, HD)
    vv = v.reshape(B * S, NUM_KV_HEADS, HD)
    for b, sq in enumerate(seqs):
        ctx, ctxp32 = sq["ctx"], sq["ctxp32"]
        npg = sq["ctxp"] // PAGE
        pages = page_tables[b, :npg].astype(np.int64)
        rows = (pages[:, None] * PAGE + np.arange(PAGE)[None, :]).reshape(-1)[:ctx]
        kasm[b, :ctx] = kcv[rows].astype(bf)
        vasm[b, :ctx] = vcv[rows].astype(bf)
        kasm[b, ctxp32 : ctxp32 + S] = kv[b * S : (b + 1) * S].astype(bf)
        vasm[b, ctxp32 : ctxp32 + S] = vv[b * S : (b + 1) * S].astype(bf)

    # device layouts: K^T [b, d, t]; V [b, t%128, tb, d]; Q^T [b, d, sg]
    kth_all = np.ascontiguousarray(kasm.transpose(2, 0, 3, 1))  # [n, B, HD, TPAD]
    vth_all = np.ascontiguousarray(
        vasm.reshape(B, NTBMAX, 128, NUM_KV_HEADS, HD).transpose(3, 0, 2, 1, 4)
    )  # [n, B, 128, NTBMAX, HD]
    qv = q.reshape(B, S, NUM_KV_HEADS, G, HD)
    qth_all = np.ascontiguousarray(
        qv.transpose(2, 0, 4, 1, 3).reshape(NUM_KV_HEADS, B, HD, SG)
    ).astype(bf)  # [n, B, HD, SG=(s,g)] ... via [n, B, d, s, g]

    in_maps = []
    for n in range(NUM_KV_HEADS):
        in_maps.append(
            {
                "kth": kth_all[n],
                "vth": vth_all[n],
                "qth": qth_all[n],
                "mh": masks,
            }
        )

    res = bass_utils.run_bass_kernel_spmd(nc, in_maps, core_ids=list(range(8)))
    global _last_results
    _last_results = res
    # per-core outh is [B, HD, SG=(s,g)] bf16; assemble [B*S, (n,g)*HD] f32
    out = np.empty((B * S, NUM_HEADS * HD), np.float32)
    ov = out.reshape(B, S, NUM_KV_HEADS, G, HD)
    for n in range(NUM_KV_HEADS):
        # [B, HD, S*G] -> [B, S, G, HD]
        on = res.results[n]["outh"].astype(np.float32).reshape(B, HD, S, G)
        ov[:, :, n, :, :] = on.transpose(0, 2, 3, 1)
    return out


_last_results = None


# revision 42
# speedup vs baseline: 1.2521x; 1.0143x over previous
# Paged sparse attention (GQA, block-masked new tokens) on 8 TRN2 NeuronCores.
#
# Sharding: tensor-parallel over the 8 KV heads (one KV head + its 4 Q heads
# per core). Every core sees all 8 sequences, so the compiled schedule
# (derived from page_tables/context_lens, identical across cores) is SPMD.
#
# Orientation: scores are computed TRANSPOSED (S^T[t, sg] per 128-row
# t-block, K^T-stationary, Q^T-moving), so the exp (ACT) writes P^T directly
# in the layout the PV matmul consumes — no probability transposes anywhere.
# Masking folds into the exp's per-partition bias (host-precomputed -1e30
# rows for the partial page / 32-alignment gap / tail pad). The softmax
# denominator comes from an extra matmul with an all-ones stationary matrix,
# which leaves the per-sg denominator replicated across all 128 PSUM
# partitions — the normalization is then a fused (OUT^T * 1/denom -> bf16)
# DVE pass, transposed back to [sg, d] by the host during the final gather.
#
# The host pre-assembles, per sequence, contiguous zero-padded K^T / V / Q^T
# buffers (page-table gather, 32-alignment gap, new tokens, tail pad all
# resolved in numpy), so every device load is one large contiguous DMA —
# no on-device DMA transposes, no gather runs, no memsets.
#
# The whole kernel is emitted as ONE flat software pipeline over t-blocks
# across all sequences: scores(i) issue ahead of PV(i-1), so at sequence
# boundaries the next sequence's first exp is never stuck behind the
# previous sequence's tail matmuls. outt/dent PSUM banks swap roles between
# consecutive sequences so the first PV of a sequence only waits on the
# previous sequence's (cheap) denominator-reciprocal read, not its full
# normalization.
#
# The block-causal mask for new tokens reduces (with sg = s*4+g ordering) to
# a suffix of valid sg columns per t-block (plus a small intra-block
# staircase zeroed on the bf16 P^T), so invalid regions are simply never
# computed. The softmax denominator streams P^T through a ones-matmul; runs
# of up to 16 qmin-equal t-blocks are pre-summed on DVE (incremental
# left-leaning bf16 accumulator, <=2 adds per pipeline step) so the
# ones-matmul touches each column once per run. Sequences are processed
# 2nd-largest first and LARGEST last: the final sequence's long context
# phase absorbs every other sequence's deferred denominator/normalize/store
# work, and its last group is a single t-block, keeping the endgame chain
# (exp -> matmul -> reciprocal -> mul -> store) short.

import sys

sys.path.insert(0, "/opt/trn_rl_repo")

import ml_dtypes
import numpy as np

B = 8
S = 256
NUM_HEADS = 32
NUM_KV_HEADS = 8
G = NUM_HEADS // NUM_KV_HEADS  # 4
HD = 128
PAGE = 16
BLOCK = 32
MAX_PAGES = 128
C = MAX_PAGES * PAGE  # 2048
SCALE = 0.08838834764831845
SG = S * G  # 1024 q rows per (seq, kv head)
TMAX = C + S + 32  # worst-case padded length
NTBMAX = (TMAX + 127) // 128
NQT = SG // 128  # 8 q-tiles per seq

NEG = -1e30


def _schedule(context_lens: np.ndarray):
    """Per-seq schedule baked into the compiled kernel (same on all cores)."""
    desc = sorted(
        range(B),
        key=lambda b: -(((int(context_lens[b]) + 15) // 16 * 16 + 31) // 32 * 32),
    )
    # 2nd-largest first (covers load latency), then descending, with the
    # LARGEST last: its long context phase absorbs every other sequence's
    # deferred denominator/close work, so almost nothing piles up after
    # the final exp
    order = desc[1:] + desc[:1]
    seqs = []
    for b in range(B):
        ctx = int(context_lens[b])
        npg = (ctx + PAGE - 1) // PAGE
        ctxp = npg * PAGE
        ctxp32 = ((ctxp + 31) // 32) * 32  # 32-align the new-token region
        ttot = ctxp32 + S
        ntb = (ttot + 127) // 128
        tq = [ctxp32 + BLOCK * (i + 1) for i in range(NQT)]
        # first valid q-tile per t-block (valid sg columns = suffix)
        qmin = [next(i for i in range(NQT) if tq[i] > tb * 128) for tb in range(ntb)]

        def fully_valid(tb):
            # every t-row in the block is a real, unmasked token
            if (tb + 1) * 128 > ttot:
                return False
            return not (ctx < (tb + 1) * 128 and tb * 128 < ctxp32)

        # denominator groups: runs of qmin-equal t-blocks pre-summed on DVE.
        # Last seq: single final group = short endgame chain.
        maxrun = 16
        dgroups = []
        tb = 0
        while tb < ntb:
            n = 1
            while n < maxrun and tb + n < ntb and qmin[tb + n] == qmin[tb]:
                n += 1
            if b == order[-1] and tb + n == ntb and n > 1:
                n -= 1  # force the final group to be a single
            dgroups.append(tuple(range(tb, tb + n)))
            tb += n
        seqs.append(
            dict(
                ctx=ctx,
                ctxp=ctxp,
                ctxp32=ctxp32,
                ttot=ttot,
                ntb=ntb,
                tq=tq,
                qmin=qmin,
                dgroups=dgroups,
                valid=[fully_valid(tb) for tb in range(ntb)],
            )
        )
    return seqs, order


def _masks(seqs):
    """Host-precomputed per-partition exp bias: [128, B, NTBMAX] fp32.
    mask[p, b, tb] is added (post-scale) to scores of t-row tb*128+p:
    0 for valid rows, -1e30 for masked rows (partial page, 32-align gap,
    padded tail)."""
    m = np.zeros((B, 128, NTBMAX), np.float32)
    for b, sq in enumerate(seqs):
        valid = np.zeros((NTBMAX * 128,), bool)
        valid[: sq["ttot"]] = True
        valid[sq["ctx"] : sq["ctxp32"]] = False  # partial page + gap
        m[b][~valid.reshape(NTBMAX, 128).T] = NEG
    return np.ascontiguousarray(m.transpose(1, 0, 2))


def _build(nc, seqs, order):
    import concourse.mybir as mybir
    import concourse.tile as tile

    bf16 = mybir.dt.bfloat16
    f32 = mybir.dt.float32

    # host-assembled, per-seq contiguous + padded, transposed layouts
    kth = nc.dram_tensor("kth", [B, HD, NTBMAX * 128], bf16, kind="ExternalInput").ap()
    vth = nc.dram_tensor("vth", [B, 128, NTBMAX, HD], bf16, kind="ExternalInput").ap()
    qth = nc.dram_tensor("qth", [B, HD, SG], bf16, kind="ExternalInput").ap()
    mh = nc.dram_tensor("mh", [128, B, NTBMAX], f32, kind="ExternalInput").ap()
    # transposed output [b, d, sg] in bf16; the host normz-reindexes
    outh = nc.dram_tensor("outh", [B, HD, SG], bf16, kind="ExternalOutput").ap()

    KSPLIT = 128  # first seq: tiny first K^T tile so matmuls start early
    chunks = ((0, 4), (4, 8))

    with tile.TileContext(nc) as tc:
        with (
            tc.tile_pool(name="cst", bufs=1) as const_pool,
            tc.tile_pool(name="kt", bufs=3) as kt_pool,
            tc.tile_pool(name="vt", bufs=3) as v_pool,
            tc.tile_pool(name="qt", bufs=3) as qt_pool,
            tc.tile_pool(name="pt", bufs=3) as pt_pool,
            tc.tile_pool(name="ds", bufs=6) as dsum_pool,
            tc.tile_pool(name="ot", bufs=2) as out_pool,
            tc.tile_pool(name="ps_s", bufs=2, space="PSUM") as psum_s,
            tc.tile_pool(name="ps_a", bufs=1, space="PSUM") as psum_a,
            tc.tile_pool(name="ps_b", bufs=1, space="PSUM") as psum_b,
        ):
            ones_t = const_pool.tile([128, 128], bf16)
            nc.vector.memset(ones_t, 1.0)

            mask_all = const_pool.tile([128, B, NTBMAX], f32)

            load_tiles = {}

            def emit_loads(b, first=False):
                sq = seqs[b]
                ntb = sq["ntb"]

                # Q^T first on the gpsimd queue: the first score matmul
                # needs only qt + the first K^T tile
                qt = qt_pool.tile([128, SG], bf16, tag="qt")
                if first:
                    nc.gpsimd.dma_start(qt[:, :512], qth[b][:, :512])
                    nc.gpsimd.dma_start(qt[:, 512:], qth[b][:, 512:])
                else:
                    nc.gpsimd.dma_start(qt, qth[b])

                # K^T on the SP queue. Only the very first seq is split
                # into two tiles (so its first matmuls start before the
                # full K^T lands).
                ksplit = KSPLIT if first else NTBMAX * 128
                ka_cols = min(ksplit, ntb * 128)
                kta = kt_pool.tile([128, ka_cols], bf16, tag="kta")
                nc.sync.dma_start(kta, kth[b][:, :ka_cols])
                if ntb * 128 > ksplit:
                    kb_cols = ntb * 128 - ksplit
                    ktb = kt_pool.tile([128, kb_cols], bf16, tag="ktb", name="ktb")
                    if first:
                        # halves: early t-blocks unblock before the full load
                        nc.sync.dma_start(
                            ktb[:, : kb_cols // 2],
                            kth[b][:, ksplit : ksplit + kb_cols // 2],
                        )
                        nc.sync.dma_start(
                            ktb[:, kb_cols // 2 :],
                            kth[b][:, ksplit + kb_cols // 2 : ntb * 128],
                        )
                    else:
                        nc.sync.dma_start(ktb, kth[b][:, ksplit : ntb * 128])
                else:
                    ktb = None

                # V natural [t%128, tb, d]; one contiguous DMA (first seq:
                # split so the first PV isn't gated on the full V load)
                vt = v_pool.tile([128, NTBMAX, HD], bf16, tag="vt")
                if first:
                    nc.gpsimd.dma_start(vt[:, :2, :], vth[b][:, :2, :])
                    nc.gpsimd.dma_start(vt[:, 2:ntb, :], vth[b][:, 2:ntb, :])
                    # masks are first needed by the exp at the ctx boundary
                    nc.gpsimd.dma_start(mask_all, mh)
                else:
                    nc.gpsimd.dma_start(vt[:, :ntb, :], vth[b][:, :ntb, :])

                load_tiles[b] = ((kta, ktb, ksplit), vt, qt)

            class Ctx:
                pass

            ctxs = {}

            def make_ctx(b, parity):
                sq = seqs[b]
                c = Ctx()
                c.b = b
                c.sq = sq
                c.kt, c.vt, c.qt = load_tiles.pop(b)
                c.mask = mask_all[:, b, :]
                c.ptt = pt_pool.tile([128, NTBMAX, SG], bf16, tag="pt", name="ptt")
                po, pd = (psum_a, psum_b) if parity == 0 else (psum_b, psum_a)
                c.outt = po.tile([128, SG], f32, tag="x", name="outt")
                c.dent = pd.tile([128, SG], f32, tag="x", name="dent")
                c.last_tb = [0, 0]
                for tb in range(sq["ntb"]):
                    for ci, (g0, g1) in enumerate(chunks):
                        if max(sq["qmin"][tb], g0) < g1:
                            c.last_tb[ci] = tb
                c.gi = 0
                c.acc = None
                c.pnext = 0
                c.nmm = [0, 0]
                c.nmm_tot = [
                    sum(1 for grp in sq["dgroups"] if max(sq["qmin"][grp[0]], g0) < g1)
                    for (g0, g1) in chunks
                ]
                c.mm_new = []
                c.mm_mid = []
                c.mm_ready = []
                c.done = -1
                c.finish_step = None
                c.closed = False
                return c

            def kt_sl(c, tb):
                kta, ktb, ksplit = c.kt
                if tb * 128 < ksplit:
                    return kta[:, tb * 128 : (tb + 1) * 128]
                return ktb[:, tb * 128 - ksplit : (tb + 1) * 128 - ksplit]

            def emit_scores(c, tb0):
                sq = c.sq
                qm = sq["qmin"][tb0]
                s_ps = psum_s.tile([128, SG], f32, tag="s", name="s_ps")
                for c0, c1 in ((qm * 128, 512), (max(512, qm * 128), SG)):
                    if c0 >= c1:
                        continue
                    nc.tensor.matmul(
                        s_ps[:, c0:c1],
                        lhsT=kt_sl(c, tb0),
                        rhs=c.qt[:, c0:c1],
                        start=True,
                        stop=True,
                    )
                nc.scalar.activation(
                    out=c.ptt[:, tb0, qm * 128 :],
                    in_=s_ps[:, qm * 128 : SG],
                    func=mybir.ActivationFunctionType.Exp,
                    scale=SCALE,
                    bias=(0.0 if sq["valid"][tb0] else c.mask[:, tb0 : tb0 + 1]),
                )
                # staircase: zero P^T rows of new-token blocks for
                # earlier q-tiles inside this t-block's suffix
                ctxp32, ttot = sq["ctxp32"], sq["ttot"]
                for r0 in range(0, 128, 32):
                    t0 = tb0 * 128 + r0
                    if t0 < ctxp32 or t0 >= ttot:
                        continue
                    blk = (t0 - ctxp32) // 32
                    if blk > qm:
                        # last seq: gpsimd (idle then) so the final PVs
                        # aren't serialized behind DVE's endgame work
                        eng = nc.gpsimd if c.b == order[-1] else nc.vector
                        eng.memset(
                            c.ptt[r0 : r0 + 32, tb0, qm * 128 : blk * 128], 0.0
                        )

            def emit_pv(c, tb):
                qm = c.sq["qmin"][tb]
                for ci, (g0, g1) in enumerate(chunks):
                    lo = max(qm, g0)
                    if lo >= g1:
                        continue
                    nc.tensor.matmul(
                        c.outt[:, lo * 128 : g1 * 128],
                        lhsT=c.vt[:, tb, :],
                        rhs=c.ptt[:, tb, lo * 128 : g1 * 128],
                        start=(tb == 0),
                        stop=(tb == c.last_tb[ci]),
                    )

            def emit_dent_mms(c, only_chunk=None):
                # emit queued ones-matmuls (their DVE adds are long done)
                keep = []
                for ci, qm, rhs_of in c.mm_ready:
                    if only_chunk is not None and ci != only_chunk:
                        keep.append((ci, qm, rhs_of))
                        continue
                    g0, g1 = chunks[ci]
                    lo = max(qm, g0)
                    nc.tensor.matmul(
                        c.dent[:, lo * 128 : g1 * 128],
                        lhsT=ones_t,
                        rhs=rhs_of(lo * 128, g1 * 128),
                        start=(c.nmm[ci] == 0),
                        stop=(c.nmm[ci] + 1 == c.nmm_tot[ci]),
                    )
                    c.nmm[ci] += 1
                c.mm_ready = keep

            def stage_new(c, ready_upto):
                # stage newly-ready groups: incremental left-leaning DVE
                # accumulator (pair-add + running join per 2 t-blocks), so
                # each pipeline step carries at most ~2 adds
                sq = c.sq
                dgroups = sq["dgroups"]
                while c.gi < len(dgroups):
                    grp = dgroups[c.gi]
                    qm = sq["qmin"][grp[0]]
                    c0 = qm * 128
                    n = len(grp)
                    ptt = c.ptt
                    while c.pnext + 1 < n and grp[c.pnext + 1] <= ready_upto:
                        t = dsum_pool.tile([128, SG], bf16, tag="ds1", name="ds1")
                        nc.vector.tensor_add(
                            t[:, c0:],
                            ptt[:, grp[c.pnext], c0:],
                            ptt[:, grp[c.pnext + 1], c0:],
                        )
                        if c.acc is None:
                            c.acc = t
                        else:
                            t2 = dsum_pool.tile([128, SG], bf16, tag="ds2", name="ds2")
                            nc.vector.tensor_add(t2[:, c0:], c.acc[:, c0:], t[:, c0:])
                            c.acc = t2
                        c.pnext += 2
                    if grp[-1] > ready_upto:
                        break
                    if n == 1:
                        rhs_of = lambda a, b, ptt=ptt, tb=grp[0]: ptt[:, tb, a:b]
                    else:
                        if c.pnext < n:  # odd run: fold in the last t-block
                            t2 = dsum_pool.tile([128, SG], bf16, tag="ds2", name="ds2")
                            nc.vector.tensor_add(
                                t2[:, c0:], c.acc[:, c0:], ptt[:, grp[-1], c0:]
                            )
                            c.acc = t2
                        rhs_of = lambda a, b, ds=c.acc: ds[:, a:b]
                        c.acc = None
                        c.pnext = 0
                    for ci, (g0, g1) in enumerate(chunks):
                        if max(qm, g0) < g1:
                            c.mm_new.append((ci, qm, rhs_of))
                    c.gi += 1

            pending_stores = []

            def emit_half(c, h, defer=True):
                # normalize one sg-half: OUT^T * (1/denom) -> bf16. The
                # store DMA is deferred one step so its wait on the mul is
                # pre-satisfied and never head-of-line-blocks the SP queue
                # (which would delay the next K^T load issue behind it).
                h0, h1 = h * (SG // 2), (h + 1) * (SG // 2)
                invh = out_pool.tile([128, SG // 2], f32, tag="invh", name="invh")
                nc.vector.reciprocal_approx_fast(invh, c.dent[:, h0:h1])
                otfh = out_pool.tile([128, SG // 2], bf16, tag="otfh", name="otfh")
                nc.vector.tensor_mul(otfh, c.outt[:, h0:h1], invh)
                if defer:
                    pending_stores.append((outh[c.b][:, h0:h1], otfh))
                else:
                    nc.sync.dma_start(outh[c.b][:, h0:h1], otfh)

            def flush_stores():
                while pending_stores:
                    dst, src = pending_stores.pop(0)
                    nc.sync.dma_start(dst, src)

            def close_seq(c):
                # per-chunk: half 0's reciprocal overlaps chunk 1's final
                # denominator matmuls on PE
                c.mm_ready += c.mm_mid + c.mm_new
                c.mm_mid = []
                c.mm_new = []
                defer = c.b != order[-1]  # last seq: store immediately
                emit_dent_mms(c, only_chunk=0)
                emit_half(c, 0, defer)
                emit_dent_mms(c, only_chunk=1)
                emit_half(c, 1, defer)
                c.closed = True

            def post(c, ptb, step):
                emit_pv(c, ptb)
                c.done = ptb
                stage_new(c, ptb)
                c.mm_ready += c.mm_mid
                c.mm_mid = c.mm_new
                c.mm_new = []
                if ptb == c.sq["ntb"] - 1:
                    c.finish_step = step

            # ---- the flat t-block pipeline across all sequences ----
            stream = [(b, tb) for b in order for tb in range(seqs[b]["ntb"])]

            emit_loads(order[0], first=True)

            # pre-warm the PE clock (HAM) with dummy matmuls while the
            # first loads are in flight
            warm_rhs = const_pool.tile([128, 512], bf16)
            nc.vector.memset(warm_rhs, 0.0)
            warm_ps = psum_s.tile([128, SG], f32, tag="s", name="s_ps")
            for _ in range(8):
                nc.tensor.matmul(
                    warm_ps[:, :512], lhsT=ones_t, rhs=warm_rhs,
                    start=True, stop=True,
                )
            warm_sink = const_pool.tile([1, 1], f32)
            nc.vector.tensor_copy(warm_sink, warm_ps[0:1, 0:1])

            emit_loads(order[1])

            seq_idx = {b: j for j, b in enumerate(order)}
            nseq = 0
            for i, (b, tb) in enumerate(stream):
                if b not in ctxs:
                    ctxs[b] = make_ctx(b, nseq % 2)
                    nseq += 1
                    j = seq_idx[b]
                    if j + 2 < B:
                        emit_loads(order[j + 2])
                emit_scores(ctxs[b], tb)
                # close any sequence finished on an EARLIER step (its final
                # DVE adds ran last step). Must precede post(): the first PV
                # of this seq writes the closed seq's recycled PSUM banks.
                for pc in list(ctxs.values()):
                    if pc.finish_step is not None and pc.finish_step < i and not pc.closed:
                        close_seq(pc)
                # PV runs at lag 2: its exp finished ~2 steps ago, so it
                # never stalls the in-order PE queue (which would delay the
                # next scores and starve ACT)
                if i >= 2:
                    pb, ptb = stream[i - 2]
                    pc = ctxs[pb]
                    # denominator matmuls staged earlier: their DVE adds
                    # are long done, so they never stall the PE queue
                    emit_dent_mms(pc)
                    post(pc, ptb, i)
                flush_stores()
            # drain the pipeline: last two t-blocks' PVs + the final close
            n = len(stream)
            for j in (n - 2, n - 1):
                pb, ptb = stream[j]
                emit_dent_mms(ctxs[pb])
                post(ctxs[pb], ptb, n)
            for pc in ctxs.values():
                if pc.finish_step is not None and not pc.closed:
                    close_seq(pc)
    return nc


def _compile(seqs, order):
    import concourse.bacc as bacc

    nc = bacc.Bacc(
        "TRN2",
        target_bir_lowering=False,
        debug=False,
        enable_asserts=False,
        num_devices=8,
    )
    _build(nc, seqs, order)
    nc.compile()
    return nc


def kernel(q, k, v, k_cache, v_cache, page_tables, context_lens, page_size, block_size, **_):
    from concourse import bass_utils

    q = np.asarray(q)
    k = np.asarray(k)
    v = np.asarray(v)
    k_cache = np.asarray(k_cache)
    v_cache = np.asarray(v_cache)
    page_tables = np.asarray(page_tables)
    context_lens = np.asarray(context_lens)
    assert int(page_size) == PAGE and int(block_size) == BLOCK
    assert q.shape == (B * S, NUM_HEADS * HD)
    assert page_tables.shape == (B, MAX_PAGES)

    seqs, order = _schedule(context_lens)
    nc = _compile(seqs, order)

    bf = ml_dtypes.bfloat16
    masks = _masks(seqs)

    # host-side assembly: per-seq contiguous padded K/V in [t, n, d] layout
    TPAD = NTBMAX * 128
    kasm = np.zeros((B, TPAD, NUM_KV_HEADS, HD), bf)
    vasm = np.zeros((B, TPAD, NUM_KV_HEADS, HD), bf)
    kcv = k_cache.reshape(MAX_PAGES * B * PAGE, NUM_KV_HEADS, HD)
    vcv = v_cache.reshape(MAX_PAGES * B * PAGE, NUM_KV_HEADS, HD)
    kv = k.reshape(B * S, NUM_KV_HEADS, HD)
    vv = v.reshape(B * S, NUM_KV_HEADS, HD)
    for b, sq in enumerate(seqs):
        ctx, ctxp32 = sq["ctx"], sq["ctxp32"]
        npg = sq["ctxp"] // PAGE
        pages = page_tables[b, :npg].astype(np.int64)
        rows = (pages[:, None] * PAGE + np.arange(PAGE)[None, :]).reshape(-1)[:ctx]
        kasm[b, :ctx] = kcv[rows].astype(bf)
        vasm[b, :ctx] = vcv[rows].astype(bf)
        kasm[b, ctxp32 : ctxp32 + S] = kv[b * S : (b + 1) * S].astype(bf)
        vasm[b, ctxp32 : ctxp32 + S] = vv[b * S : (b + 1) * S].astype(bf)

    # device layouts: K^T [b, d, t]; V [b, t%128, tb, d]; Q^T [b, d, sg]
    kth_all = np.ascontiguousarray(kasm.transpose(2, 0, 3, 1))  # [n, B, HD, TPAD]
    vth_all = np.ascontiguousarray(
        vasm.reshape(B, NTBMAX, 128, NUM_KV_HEADS, HD).transpose(3, 0, 2, 1, 4)
    )  # [n, B, 128, NTBMAX, HD]
    qv = q.reshape(B, S, NUM_KV_HEADS, G, HD)
    qth_all = np.ascontiguousarray(
        qv.transpose(2, 0, 4, 1, 3).reshape(NUM_KV_HEADS, B, HD, SG)
    ).astype(bf)  # [n, B, HD, SG=(s,g)] ... via [n, B, d, s, g]

    in_maps = []
    for n in range(NUM_KV_HEADS):
        in_maps.append(
            {
                "kth": kth_all[n],
                "vth": vth_all[n],
                "qth": qth_all[n],
                "mh": masks,
            }
        )

    res = bass_utils.run_bass_kernel_spmd(nc, in_maps, core_ids=list(range(8)))
    global _last_results
    _last_results = res
    # per-core outh is [B, HD, SG=(s,g)] bf16; assemble [B*S, (n,g)*HD] f32
    out = np.empty((B * S, NUM_HEADS * HD), np.float32)
    ov = out.reshape(B, S, NUM_KV_HEADS, G, HD)
    for n in range(NUM_KV_HEADS):
        # [B, HD, S*G] -> [B, S, G, HD]
        on = res.results[n]["outh"].astype(np.float32).reshape(B, HD, S, G)
        ov[:, :, n, :, :] = on.transpose(0, 2, 3, 1)
    return out


_last_results = None
